# revision 8
# baseline (speedup 1.0000x reference)
"""MoE gate (router) kernel for Trainium2.

Computes, for hidden_states [T, H] and gate weight [E, H]:
    logits = hidden_states @ weight.T          # [T, E]
    probs  = softmax(logits, axis=-1)
    topk_weight, topk_idx = top_k(probs, 8)    # normalized over the top-8
    row_idx = arange(T*8).reshape(8, T).T

Strategy (8 NeuronCores, data parallel over tokens):
  - Host pre-transposes: each core receives hsT [H, T/8] and wT [H, E] so the
    contraction dim H lands on SBUF partitions with fully-contiguous DMA --
    no on-device transposes at all.
  - fp32 accuracy via bf16 hi/lo split (done on host, same DMA bytes as f32):
    x = hi + lo with hi = bf16(x), lo = bf16(x - hi).  Each k-tile does three
    bf16 matmuls (hi*hi + hi*lo + lo*hi) accumulating into fp32 PSUM; the
    dropped lo*lo term is ~2^-18 relative.  This is both more precise per
    cycle than native fp32 matmul (4 cycles/row) and avoids the walrus
    codegen limit on sync waits for self-loading fp32 LDWEIGHTS.
  - Per 128-token tile: logits [128t, 256e] accumulate directly in PSUM
    (lhsT = hs k-chunk [128h x 128t], rhs = wT k-chunk [128h x 256e]).
  - DVE max/max_index give the top-8 values + indices per token in one
    instruction each.  Softmax over the full 256 experts followed by top-k
    renormalization reduces algebraically to a softmax over just the top-8
    logits, so the full-row softmax is never materialized.
"""

import numpy as np

TOP_K = 8
NUM_EXPERTS = 256
HIDDEN = 7168
NUM_TOKENS = 16384
N_CORES = 8
T_LOC = NUM_TOKENS // N_CORES

_NC_CACHE = {}


def build_gate_nc(t_loc=T_LOC, h=HIDDEN, e=NUM_EXPERTS):
    import concourse.mybir as mybir
    import concourse.tile as tile
    from concourse import bacc

    f32 = mybir.dt.float32
    bf16 = mybir.dt.bfloat16
    P = 128
    KT = h // P          # k-tiles along hidden dim
    TS = t_loc // P      # 128-token subtiles per core
    KC = 8 if KT % 8 == 0 else (4 if KT % 4 == 0 else 1)  # k-tiles per DMA
    NKC = KT // KC       # number of k-chunks

    nc = bacc.Bacc("TRN2", target_bir_lowering=False)
    hsT_hi = nc.dram_tensor("hsT_hi", [h, t_loc], bf16, kind="ExternalInput")
    hsT_lo = nc.dram_tensor("hsT_lo", [h, t_loc], bf16, kind="ExternalInput")
    wT_hi = nc.dram_tensor("wT_hi", [h, e], bf16, kind="ExternalInput")
    wT_lo = nc.dram_tensor("wT_lo", [h, e], bf16, kind="ExternalInput")
    idx_out = nc.dram_tensor(
        "topk_idx", [t_loc, TOP_K], mybir.dt.int32, kind="ExternalOutput"
    )
    w_out = nc.dram_tensor("topk_w", [t_loc, TOP_K], f32, kind="ExternalOutput")

    # [128, KT, t_loc] / [128, KT, e] views with H split over partitions
    hshi_t = hsT_hi[:, :].rearrange("(ko p) t -> p ko t", p=P)
    hslo_t = hsT_lo[:, :].rearrange("(ko p) t -> p ko t", p=P)
    wthi_t = wT_hi[:, :].rearrange("(ko p) e -> p ko e", p=P)
    wtlo_t = wT_lo[:, :].rearrange("(ko p) e -> p ko e", p=P)

    with tile.TileContext(nc) as tc:
        with (
            tc.tile_pool(name="wpool", bufs=1) as wpool,
            tc.tile_pool(name="hpool", bufs=28) as hpool,
            tc.tile_pool(name="lpool", bufs=3) as lpool,
            tc.tile_pool(name="spool", bufs=4) as spool,
            tc.tile_pool(name="opool", bufs=4) as opool,
            tc.tile_pool(name="psum", bufs=4, space="PSUM") as psum_pool,
        ):
            # output staging: small per-tile results accumulate here and leave
            # as two large descriptor DMAs at the end (tiny per-tile DMAs get
            # the DIRECT2D encoding whose single wait slot walrus overflows)
            stage_idx = wpool.tile([P, TS, TOP_K], mybir.dt.int32, tag="sidx")
            stage_wv = wpool.tile([P, TS, TOP_K], f32, tag="swv")
            # gate weight: resident in SBUF, one tile per k-chunk so each
            # matmul depends on exactly one weight-load DMA
            wt_chunks = []
            for kc in range(NKC):
                whi = wpool.tile([P, KC, e], bf16, tag=f"wthi{kc}", name=f"wthi{kc}")
                nc.sync.dma_start(whi, wthi_t[:, kc * KC : (kc + 1) * KC, :])
                wlo = wpool.tile([P, KC, e], bf16, tag=f"wtlo{kc}", name=f"wtlo{kc}")
                nc.sync.dma_start(wlo, wtlo_t[:, kc * KC : (kc + 1) * KC, :])
                wt_chunks.append((whi, wlo))
            for ts_i in range(TS):
                tslc = slice(ts_i * P, (ts_i + 1) * P)
                hs_chunks = []
                for kc in range(NKC):
                    kslc = slice(kc * KC, (kc + 1) * KC)
                    hhi = hpool.tile([P, KC, P], bf16, tag="hs", name=f"hshi{ts_i}_{kc}")
                    nc.sync.dma_start(hhi, hshi_t[:, kslc, tslc])
                    hlo = hpool.tile([P, KC, P], bf16, tag="hs", name=f"hslo{ts_i}_{kc}")
                    nc.sync.dma_start(hlo, hslo_t[:, kslc, tslc])
                    hs_chunks.append((hhi, hlo))
                pt = psum_pool.tile([P, e], f32, tag="pt")
                n_mm = 3 * KT
                mm_i = 0
                for k in range(KT):
                    kc, ki = divmod(k, KC)
                    hhi, hlo = hs_chunks[kc]
                    whi, wlo = wt_chunks[kc]
                    for lhsT, rhs in (
                        (hhi, whi),
                        (hhi, wlo),
                        (hlo, whi),
                    ):
                        nc.tensor.matmul(
                            pt,
                            lhsT[:, ki, :],
                            rhs[:, ki, :],
                            start=(mm_i == 0),
                            stop=(mm_i == n_mm - 1),
                        )
                        mm_i += 1
                logits = lpool.tile([P, e], f32, tag="logits")
                nc.vector.tensor_copy(logits, pt)
                mx = spool.tile([P, TOP_K], f32, tag="mx")
                nc.vector.max(out=mx, in_=logits)
                idx_u = spool.tile([P, TOP_K], mybir.dt.uint32, tag="idxu")
                nc.vector.max_index(idx_u, mx, logits)
                nc.vector.tensor_copy(stage_idx[:, ts_i, :], idx_u)
                # normalized top-k softmax: exp(v - v_max) / sum
                nm = spool.tile([P, 1], f32, tag="nm")
                nc.vector.tensor_scalar_mul(nm, mx[:, 0:1], -1.0)
                ev = spool.tile([P, TOP_K], f32, tag="ev")
                sm = spool.tile([P, 1], f32, tag="sm")
                nc.scalar.activation(
                    ev,
                    mx,
                    mybir.ActivationFunctionType.Exp,
                    bias=nm,
                    scale=1.0,
                    accum_out=sm,
                )
                rc = spool.tile([P, 1], f32, tag="rc")
                nc.vector.reciprocal(rc, sm)
                nc.vector.tensor_scalar_mul(stage_wv[:, ts_i, :], ev, rc)
            nc.sync.dma_start(
                idx_out[:, :].rearrange("(ts p) k -> p ts k", p=P), stage_idx
            )
            nc.sync.dma_start(
                w_out[:, :].rearrange("(ts p) k -> p ts k", p=P), stage_wv
            )
    nc.compile()
    return nc


def _get_nc():
    key = (T_LOC, HIDDEN, NUM_EXPERTS)
    if key not in _NC_CACHE:
        _NC_CACHE[key] = build_gate_nc(*key)
    return _NC_CACHE[key]


def _split_bf16(x):
    """x (f32) -> (hi, lo) bf16 with hi + lo ~= x to ~2^-17 relative."""
    import ml_dtypes

    hi = x.astype(ml_dtypes.bfloat16)
    lo = (x - hi.astype(np.float32)).astype(ml_dtypes.bfloat16)
    return hi, lo


def kernel(hidden_states, weight):
    from concourse.bass_utils import run_bass_kernel_spmd

    hs = np.asarray(hidden_states, dtype=np.float32)
    w = np.asarray(weight, dtype=np.float32)
    assert hs.shape == (NUM_TOKENS, HIDDEN), hs.shape
    assert w.shape == (NUM_EXPERTS, HIDDEN), w.shape

    wT = np.ascontiguousarray(w.T)  # [H, E]
    wT_hi, wT_lo = _split_bf16(wT)
    in_maps = []
    for c in range(N_CORES):
        hsT_c = np.ascontiguousarray(hs[c * T_LOC : (c + 1) * T_LOC].T)  # [H, T_LOC]
        hsT_hi, hsT_lo = _split_bf16(hsT_c)
        in_maps.append(
            {"hsT_hi": hsT_hi, "hsT_lo": hsT_lo, "wT_hi": wT_hi, "wT_lo": wT_lo}
        )

    nc = _get_nc()
    res = run_bass_kernel_spmd(nc, in_maps, core_ids=list(range(N_CORES)))

    topk_idx = np.concatenate([r["topk_idx"] for r in res.results], axis=0)
    topk_w = np.concatenate([r["topk_w"] for r in res.results], axis=0)
    row_idx = (
        np.arange(NUM_TOKENS * TOP_K, dtype=np.int32).reshape(TOP_K, NUM_TOKENS).T
    )
    return (
        topk_idx.astype(np.int32),
        topk_w.astype(np.float32),
        row_idx,
    )


# revision 9
# speedup vs baseline: 5.7066x; 5.7066x over previous
"""MoE gate (router) kernel for Trainium2.

Computes, for hidden_states [T, H] and gate weight [E, H]:
    logits = hidden_states @ weight.T          # [T, E]
    probs  = softmax(logits, axis=-1)
    topk_weight, topk_idx = top_k(probs, 8)    # normalized over the top-8
    row_idx = arange(T*8).reshape(8, T).T

Strategy (8 NeuronCores, data parallel over tokens):
  - Host pre-transposes: each core receives hsT [H, T/8] and wT [H, E] so the
    contraction dim H lands on SBUF partitions with fully-contiguous DMA --
    no on-device transposes at all.
  - fp32 accuracy from fp16 hi/lo splits (host-side, same DMA bytes as f32):
    hs = hi + lo/2^11, 64*w = whi + wlo/2^11, with each part fp16 (11-bit
    mantissa, so ~22 mantissa bits total; the dropped lo*lo term is ~2^-22).
    The scaling keeps the lo parts in fp16 normal range.  Native fp32 matmul
    would be 4 cycles/row and trips a walrus codegen limit on sync waits for
    self-loading fp32 LDWEIGHTS; fp16 runs 1 cycle/row.
  - Per k-tile only TWO matmuls: rhs = [whi | wlo] concatenated [128 x 512]
    shares one weight load for the hi*hi and hi*lo terms; the lo*hi term
    accumulates into the same scaled-2^11 PSUM columns as hi*lo:
        psum[:, 0:256]   += hshi . whi
        psum[:, 256:512] += hshi . wlo + hslo . whi
    logits = 2^-6 * psum[:, 0:256] + 2^-17 * psum[:, 256:512]
  - DVE max/max_index give the top-8 values + indices per token in one
    instruction each.  Softmax over the full 256 experts followed by top-k
    renormalization reduces algebraically to a softmax over just the top-8
    logits, so the full-row softmax is never materialized.
"""

import numpy as np

TOP_K = 8
NUM_EXPERTS = 256
HIDDEN = 7168
NUM_TOKENS = 16384
N_CORES = 8
T_LOC = NUM_TOKENS // N_CORES

W_SCALE = 64.0       # weight pre-scale so fp16(64*w) stays normal-range
LO_SCALE = 2048.0    # 2^11: lo parts carry the next 11 mantissa bits

_NC_CACHE = {}


def build_gate_nc(t_loc=T_LOC, h=HIDDEN, e=NUM_EXPERTS, repeat=1):
    import concourse.mybir as mybir
    import concourse.tile as tile
    from concourse import bacc

    f32 = mybir.dt.float32
    fp16 = mybir.dt.float16
    P = 128
    KT = h // P          # k-tiles along hidden dim
    TS = t_loc // P      # 128-token subtiles per core
    KC = 8 if KT % 8 == 0 else (4 if KT % 4 == 0 else 1)  # k-tiles per DMA
    NKC = KT // KC       # number of k-chunks

    nc = bacc.Bacc("TRN2", target_bir_lowering=False)
    hsT_hi = nc.dram_tensor("hsT_hi", [h, t_loc], fp16, kind="ExternalInput")
    hsT_lo = nc.dram_tensor("hsT_lo", [h, t_loc], fp16, kind="ExternalInput")
    # wT_cat[:, 0:e] = fp16(64*wT), wT_cat[:, e:2e] = fp16((64*wT - hi) * 2^11)
    wT_cat = nc.dram_tensor("wT_cat", [h, 2 * e], fp16, kind="ExternalInput")
    idx_out = nc.dram_tensor(
        "topk_idx", [t_loc, TOP_K], mybir.dt.int32, kind="ExternalOutput"
    )
    w_out = nc.dram_tensor("topk_w", [t_loc, TOP_K], f32, kind="ExternalOutput")

    # [128, KT, *] views with H split over partitions
    hshi_t = hsT_hi[:, :].rearrange("(ko p) t -> p ko t", p=P)
    hslo_t = hsT_lo[:, :].rearrange("(ko p) t -> p ko t", p=P)
    wcat_t = wT_cat[:, :].rearrange("(ko p) e -> p ko e", p=P)

    with tile.TileContext(nc) as tc:
        with (
            tc.tile_pool(name="wpool", bufs=1) as wpool,
            tc.tile_pool(name="hpool", bufs=28) as hpool,
            tc.tile_pool(name="lpool", bufs=3) as lpool,
            tc.tile_pool(name="spool", bufs=4) as spool,
            tc.tile_pool(name="psum", bufs=4, space="PSUM") as psum_pool,
        ):
            # output staging: small per-tile results accumulate here and leave
            # as two large descriptor DMAs at the end (tiny per-tile DMAs get
            # the DIRECT2D encoding whose single wait slot walrus overflows)
            stage_idx = wpool.tile([P, TS, TOP_K], mybir.dt.int32, tag="sidx")
            stage_wv = wpool.tile([P, TS, TOP_K], f32, tag="swv")
            # gate weight: resident in SBUF, one tile per k-chunk so each
            # matmul depends on exactly one weight-load DMA
            wt_chunks = []
            for kc in range(NKC):
                wc = wpool.tile([P, KC, 2 * e], fp16, tag=f"wt{kc}", name=f"wt{kc}")
                nc.sync.dma_start(wc, wcat_t[:, kc * KC : (kc + 1) * KC, :])
                wt_chunks.append(wc)
            for rep in range(repeat):
                for ts_i in range(TS):
                    tslc = slice(ts_i * P, (ts_i + 1) * P)
                    hs_chunks = []
                    for kc in range(NKC):
                        kslc = slice(kc * KC, (kc + 1) * KC)
                        hhi = hpool.tile(
                            [P, KC, P], fp16, tag="hs", name=f"hshi{rep}_{ts_i}_{kc}"
                        )
                        nc.sync.dma_start(hhi, hshi_t[:, kslc, tslc])
                        hlo = hpool.tile(
                            [P, KC, P], fp16, tag="hs", name=f"hslo{rep}_{ts_i}_{kc}"
                        )
                        nc.sync.dma_start(hlo, hslo_t[:, kslc, tslc])
                        hs_chunks.append((hhi, hlo))
                    pt = psum_pool.tile([P, 2 * e], f32, tag="pt")
                    for k in range(KT):
                        kc, ki = divmod(k, KC)
                        hhi, hlo = hs_chunks[kc]
                        wc = wt_chunks[kc]
                        # psum[:, 0:2e] += hshi . [whi | wlo]
                        nc.tensor.matmul(
                            pt,
                            hhi[:, ki, :],
                            wc[:, ki, :],
                            start=(k == 0),
                            stop=False,
                        )
                        # psum[:, e:2e] += hslo . whi   (same 2^11 scale as hi*lo)
                        nc.tensor.matmul(
                            pt[:, e:],
                            hlo[:, ki, :],
                            wc[:, ki, :e],
                            start=False,
                            stop=(k == KT - 1),
                        )
                    # logits = 2^-6 * psum_hi + 2^-17 * psum_cross
                    cross = lpool.tile([P, e], f32, tag="cross")
                    nc.vector.tensor_scalar_mul(cross, pt[:, e:], 1.0 / (64.0 * 2048.0))
                    logits = lpool.tile([P, e], f32, tag="logits")
                    nc.vector.tensor_scalar(
                        logits,
                        pt[:, :e],
                        1.0 / 64.0,
                        None,
                        mybir.AluOpType.mult,
                    )
                    nc.vector.tensor_add(logits, logits, cross)
                    mx = spool.tile([P, TOP_K], f32, tag="mx")
                    nc.vector.max(out=mx, in_=logits)
                    idx_u = spool.tile([P, TOP_K], mybir.dt.uint32, tag="idxu")
                    nc.vector.max_index(idx_u, mx, logits)
                    nc.vector.tensor_copy(stage_idx[:, ts_i, :], idx_u)
                    # normalized top-k softmax: exp(v - v_max) / sum
                    nm = spool.tile([P, 1], f32, tag="nm")
                    nc.vector.tensor_scalar_mul(nm, mx[:, 0:1], -1.0)
                    ev = spool.tile([P, TOP_K], f32, tag="ev")
                    sm = spool.tile([P, 1], f32, tag="sm")
                    nc.scalar.activation(
                        ev,
                        mx,
                        mybir.ActivationFunctionType.Exp,
                        bias=nm,
                        scale=1.0,
                        accum_out=sm,
                    )
                    rc = spool.tile([P, 1], f32, tag="rc")
                    nc.vector.reciprocal(rc, sm)
                    nc.vector.tensor_scalar_mul(stage_wv[:, ts_i, :], ev, rc)
            nc.sync.dma_start(
                idx_out[:, :].rearrange("(ts p) k -> p ts k", p=P), stage_idx
            )
            nc.sync.dma_start(
                w_out[:, :].rearrange("(ts p) k -> p ts k", p=P), stage_wv
            )
    nc.compile()
    return nc


def _get_nc():
    key = (T_LOC, HIDDEN, NUM_EXPERTS)
    if key not in _NC_CACHE:
        _NC_CACHE[key] = build_gate_nc(*key)
    return _NC_CACHE[key]


def _split_fp16(x, pre_scale=1.0):
    """x (f32) -> (hi, lo) fp16 with hi + lo/2^11 ~= pre_scale*x."""
    xs = x * np.float32(pre_scale) if pre_scale != 1.0 else x
    hi = xs.astype(np.float16)
    lo = ((xs - hi.astype(np.float32)) * np.float32(LO_SCALE)).astype(np.float16)
    return hi, lo


def _prep_inputs(hs, w):
    wT = np.ascontiguousarray(w.T)  # [H, E]
    w_hi, w_lo = _split_fp16(wT, W_SCALE)
    wT_cat = np.concatenate([w_hi, w_lo], axis=1)  # [H, 2E]
    in_maps = []
    for c in range(N_CORES):
        hsT_c = np.ascontiguousarray(hs[c * T_LOC : (c + 1) * T_LOC].T)  # [H, T_LOC]
        hs_hi, hs_lo = _split_fp16(hsT_c)
        in_maps.append({"hsT_hi": hs_hi, "hsT_lo": hs_lo, "wT_cat": wT_cat})
    return in_maps


def kernel(hidden_states, weight):
    from concourse.bass_utils import run_bass_kernel_spmd

    hs = np.asarray(hidden_states, dtype=np.float32)
    w = np.asarray(weight, dtype=np.float32)
    assert hs.shape == (NUM_TOKENS, HIDDEN), hs.shape
    assert w.shape == (NUM_EXPERTS, HIDDEN), w.shape

    in_maps = _prep_inputs(hs, w)
    nc = _get_nc()
    res = run_bass_kernel_spmd(nc, in_maps, core_ids=list(range(N_CORES)))

    topk_idx = np.concatenate([r["topk_idx"] for r in res.results], axis=0)
    topk_w = np.concatenate([r["topk_w"] for r in res.results], axis=0)
    row_idx = (
        np.arange(NUM_TOKENS * TOP_K, dtype=np.int32).reshape(TOP_K, NUM_TOKENS).T
    )
    return (
        topk_idx.astype(np.int32),
        topk_w.astype(np.float32),
        row_idx,
    )
